# revision 10
# baseline (speedup 1.0000x reference)
"""ISTA column-update kernel for Trainium2 (8 NeuronCores, Bass/Tile).

Computes, for k = idx+1 columns:
    resid = Y[:, :k] - phi @ X[:, :k]
    upd   = X[:, :k] + step * (phi.T @ resid)
    out   = hardshrink(upd, 0.1);  X[:, :k] = out

Sharding: columns split into 8 contiguous chunks (data parallel, no comms).
phi is tiny [64, 256] and replicated; the host pre-transposes it (cst) for
the first GEMM's stationary operand and pre-scales it by step (phs) for the
second GEMM so the step multiply costs nothing on device.

All per-tile inputs are host-packed into ONE DRAM tensor so each tile is a
single 128-partition DMA in and one out (the Matmult/TensorTensor hardware
structs only fit 1-2 sync waits, so fewer DMA queues touching each consumer
is also a correctness requirement, not just a perf nicety).

Packed layouts per column tile t of T columns (T2 = T/2):
    IN [128, NT*(2T+T2)]:  [ X[0:128] | X[128:256] | Ypk ]  per tile
        where Ypk[0:64]   = Y[:, 0:T2] (tile-local cols)
              Ypk[64:128] = Y[:, T2:T]
    OUT [128, NT*2T]:      [ upd[0:128] | upd[128:256] ]    per tile
"""

import numpy as np

M_PHI = 64
N_DICT = 256
THRESH = 0.1
N_CORES = 8
T_TILE = 512

_CACHE = {}


def _build_nc(C, T):
    import concourse.bacc as bacc
    import concourse.mybir as mybir
    import concourse.tile as tile

    f32 = mybir.dt.float32
    NT = C // T
    assert C % T == 0 and T % 2 == 0
    T2 = T // 2
    IW = 2 * T + T2
    OW = 2 * T

    nc = bacc.Bacc("TRN2")
    CST = nc.dram_tensor("CST", [128, 128], f32, kind="ExternalInput")
    PHS = nc.dram_tensor("PHS", [M_PHI, N_DICT], f32, kind="ExternalInput")
    IN = nc.dram_tensor("IN", [128, NT * IW], f32, kind="ExternalInput")
    OUT = nc.dram_tensor("OUT", [128, NT * OW], f32, kind="ExternalOutput")

    with tile.TileContext(nc) as tc:
        with (
            tc.tile_pool(name="const", bufs=1) as const,
            tc.tile_pool(name="sbuf", bufs=3) as sbuf,
            tc.tile_pool(name="ps1", bufs=2, space="PSUM") as ps1,
            tc.tile_pool(name="ps2", bufs=2, space="PSUM") as ps2,
        ):
            cst = const.tile([128, 128], f32, tag="cst")
            nc.sync.dma_start(cst[:], CST[:])
            phs = const.tile([M_PHI, N_DICT], f32, tag="phs")
            nc.sync.dma_start(phs[:], PHS[:])

            # Warmup matmuls: consume each const tile's DMA wait on the PE
            # before the loop (the Matmult LDWEIGHTS struct fits one wait).
            warm = ps1.tile([128, 1], f32, tag="warm")
            nc.tensor.matmul(warm[0:64, :], cst[:, 0:64], cst[:, 0:1],
                             start=True, stop=True)
            nc.tensor.matmul(warm[:, :], phs[:, 0:128], phs[:, 0:1],
                             start=True, stop=True)

            for t in range(NT):
                it = sbuf.tile([128, IW], f32, tag="it")
                nc.sync.dma_start(it[:], IN[:, t * IW : (t + 1) * IW])

                p1 = ps1.tile([M_PHI, T], f32, tag="p1")
                nc.tensor.matmul(p1[:], cst[:, 0:64], it[:, 0:T],
                                 start=True, stop=False)
                nc.tensor.matmul(p1[:], cst[:, 64:128], it[:, T : 2 * T],
                                 start=False, stop=True)

                rs = sbuf.tile([M_PHI, T], f32, tag="rs")
                nc.vector.tensor_sub(
                    rs[:, 0:T2], it[0:64, 2 * T : 2 * T + T2], p1[:, 0:T2]
                )
                nc.vector.tensor_sub(
                    rs[:, T2:T], it[64:128, 2 * T : 2 * T + T2], p1[:, T2:T]
                )

                p2 = ps2.tile([128, 2 * T], f32, tag="p2")
                nc.tensor.matmul(p2[:, 0:T], phs[:, 0:128], rs[:],
                                 start=True, stop=True)
                nc.tensor.matmul(p2[:, T : 2 * T], phs[:, 128:256], rs[:],
                                 start=True, stop=True)

                # upd = x + step*phi.T@resid; hardshrink via |w|>l <=> w^2>l^2
                wab = sbuf.tile([128, 2 * T], f32, tag="wab")
                sq = sbuf.tile([128, 2 * T], f32, tag="sq")
                uab = sbuf.tile([128, 2 * T], f32, tag="uab")
                nc.vector.tensor_add(wab[:], it[:, 0 : 2 * T], p2[:])
                nc.scalar.activation(
                    sq[:], wab[:], mybir.ActivationFunctionType.Square
                )
                nc.vector.scalar_tensor_tensor(
                    uab[:], sq[:], THRESH * THRESH, wab[:],
                    mybir.AluOpType.is_gt, mybir.AluOpType.mult,
                )
                nc.sync.dma_start(OUT[:, t * OW : (t + 1) * OW], uab[:])

    nc.compile()
    return nc


def _get_nc(C, T):
    key = (C, T)
    if key not in _CACHE:
        _CACHE[key] = _build_nc(C, T)
    return _CACHE[key]


def _pack_inputs(Xc, Yc, T):
    """Pack one core's X [256, C] and Y [64, C] chunks into IN [128, NT*IW]."""
    C = Xc.shape[1]
    NT = C // T
    T2 = T // 2
    IW = 2 * T + T2
    IN = np.empty((128, NT * IW), dtype=np.float32)
    in_r = IN.reshape(128, NT, IW)
    x_r = Xc.reshape(2, 128, NT, T)
    in_r[:, :, 0:T] = x_r[0]
    in_r[:, :, T : 2 * T] = x_r[1]
    y_r = Yc.reshape(M_PHI, NT, 2, T2)
    in_r[0:64, :, 2 * T : 2 * T + T2] = y_r[:, :, 0, :]
    in_r[64:128, :, 2 * T : 2 * T + T2] = y_r[:, :, 1, :]
    return IN


def _unpack_output(OUT, T):
    """Unpack OUT [128, NT*2T] back to upd [256, C]."""
    NT = OUT.shape[1] // (2 * T)
    C = NT * T
    o_r = OUT.reshape(128, NT, 2, T)
    upd = np.empty((N_DICT, C), dtype=np.float32)
    u_r = upd.reshape(2, 128, NT, T)
    u_r[0] = o_r[:, :, 0, :]
    u_r[1] = o_r[:, :, 1, :]
    return upd


def run_sharded(phi, X, Y, step, k, trace=False):
    """Run the device kernel over the first k columns; returns (upd[:, :k], results)."""
    from concourse.bass_utils import run_bass_kernel_spmd

    T = T_TILE
    C = -(-k // (N_CORES * T)) * T  # per-core columns, padded to tile multiple
    Cp = N_CORES * C

    phi = np.asarray(phi, dtype=np.float32)
    step_v = np.float32(np.asarray(step).reshape(-1)[0])
    phiT = np.ascontiguousarray(phi.T)          # [256, 64]
    phiS = phi * step_v                          # [64, 256]
    CSTh = np.empty((128, 128), dtype=np.float32)
    CSTh[:, 0:64] = phiT[0:128, :]
    CSTh[:, 64:128] = phiT[128:256, :]

    Xp = np.zeros((N_DICT, Cp), dtype=np.float32)
    Xp[:, :k] = np.asarray(X[:, :k], dtype=np.float32)
    Yp = np.zeros((M_PHI, Cp), dtype=np.float32)
    Yp[:, :k] = np.asarray(Y[:, :k], dtype=np.float32)

    nc = _get_nc(C, T)
    in_maps = [
        {
            "CST": CSTh,
            "PHS": np.ascontiguousarray(phiS),
            "IN": _pack_inputs(
                Xp[:, i * C : (i + 1) * C], Yp[:, i * C : (i + 1) * C], T
            ),
        }
        for i in range(N_CORES)
    ]
    res = run_bass_kernel_spmd(nc, in_maps, list(range(N_CORES)), trace=trace)
    upd = np.concatenate(
        [_unpack_output(np.asarray(r["OUT"]), T) for r in res.results], axis=1
    )[:, :k]
    return upd, res


def kernel(phi, X, Y, step, idx):
    k = int(idx) + 1
    upd, _ = run_sharded(phi, X, Y, step, k)
    Xout = np.array(np.asarray(X, dtype=np.float32))
    Xout[:, :k] = upd
    return np.asarray(phi, dtype=np.float32), Xout
